# revision 6
# baseline (speedup 1.0000x reference)
"""Trainium2 Bass kernel for nn_MeshGraphBlock (GNN message-passing block).

Computes, for x:[B,N,D], edges (src,dst):[E], degree:[N]:
    neighbor = scatter_add(x[:, src, :] -> dst) / clip(degree, 1)
    h  = concat(LN(x; sn_g, sn_b), LN(neighbor; nn_g, nn_b))   # [B,N,2D]
    h  = gelu_erf(h @ W1 + b1)                                  # [B,N,2D]
    y  = x + h @ W2 + b2                                        # [B,N,D]

Strategy (8 NeuronCores, SPMD):
 - Destination-node tiles (128 nodes each) are assigned to cores via
   sorted round-robin so every core sees the same per-position edge-tile
   counts (the single compiled program is uniform; only data differs).
 - Host pre-sorts edges by dst, packs x (both batches side by side) as a
   bf16 [N,2D] table, and emits per-core gather indices (int16, split in
   two tables to stay under the 32767 index limit).
 - On device, edge messages are gathered with dma_gather (512B rows) and
   scatter-added into 128-dst PSUM accumulators via one-hot "selection
   matrix" matmuls (S[e,dst] built on DVE with iota==dst_local compares).
 - LayerNorm gamma/beta are folded into W1/b1 on device, LN itself uses
   bn_stats/bn_aggr; the MLP runs as PE matmuls with PE transposes to move
   between node-major and feature-major layouts.
"""

import math

import numpy as np
import ml_dtypes

P = 128
NCORES = 8
SPLIT = 32768          # int16 gather-index limit
MAX_TILES_PER_CALL = 8  # 1024 idxs per dma_gather (SWDGE ring limit)
SCHUNK = 8              # selection-matrix tiles built per DVE op

_CACHE = {}


def _prep(x, edge_src, edge_dst, degree):
    """Host-side sharding. Returns (structure, per-core inputs, assembly map)."""
    Bb, N, D = x.shape
    E = edge_src.shape[0]
    es = np.asarray(edge_src).astype(np.int64).ravel()
    ed = np.asarray(edge_dst).astype(np.int64).ravel()
    deg = np.asarray(degree).astype(np.float32).ravel()

    ntiles = math.ceil(N / P)
    ntiles_pad = math.ceil(ntiles / NCORES) * NCORES
    NTC = ntiles_pad // NCORES

    order = np.argsort(ed, kind="stable")
    ed_s = ed[order]
    es_s = es[order]
    bounds = np.searchsorted(ed_s, np.arange(ntiles_pad + 1) * P)

    counts = bounds[1:] - bounds[:-1]
    ranked = np.argsort(-counts, kind="stable")
    # tile ranked[i] -> core i % 8, position i // 8
    tids = [[0] * NTC for _ in range(NCORES)]
    for i, t in enumerate(ranked):
        tids[i % NCORES][i // NCORES] = int(t)

    # per (core, pos): split into G0 (src < SPLIT) and G1
    g0i, g1i, dli = {}, {}, {}
    for c in range(NCORES):
        for k in range(NTC):
            t = tids[c][k]
            a, b = bounds[t], bounds[t + 1]
            srcs = es_s[a:b]
            dloc = (ed_s[a:b] - t * P).astype(np.int64)
            m0 = srcs < SPLIT
            g0i[c, k] = srcs[m0].astype(np.int64)
            g1i[c, k] = (srcs[~m0] - SPLIT).astype(np.int64)
            dli[c, k] = (dloc[m0], dloc[~m0])

    T0 = [max(math.ceil(len(g0i[c, k]) / P) for c in range(NCORES)) for k in range(NTC)]
    T1 = [max(math.ceil(len(g1i[c, k]) / P) for c in range(NCORES)) for k in range(NTC)]

    # flat per-core index/dst-local streams in position order
    TTOT = sum(T0) + sum(T1)
    idx_flat = np.zeros((NCORES, TTOT * P), dtype=np.int16)
    dl_flat = np.full((NCORES, TTOT * P), -1.0, dtype=np.float32)
    calls = []  # (pos, 'A'|'B', slot_off, ntiles, idx_off) -- uniform across cores
    tile_off = 0
    for k in range(NTC):
        slot = 0
        for grp, T in ((0, T0[k]), (1, T1[k])):
            if T == 0:
                continue
            for c in range(NCORES):
                ii = g0i[c, k] if grp == 0 else g1i[c, k]
                dd = dli[c, k][grp]
                o = tile_off * P
                idx_flat[c, o : o + len(ii)] = ii.astype(np.int16)
                dl_flat[c, o : o + len(dd)] = dd.astype(np.float32)
            nt_done = 0
            while nt_done < T:
                nt = min(T - nt_done, MAX_TILES_PER_CALL)
                calls.append(
                    (k, "A" if grp == 0 else "B", slot + nt_done,
                     nt, (tile_off + nt_done) * P)
                )
                nt_done += nt
            tile_off += T
            slot += T
    assert tile_off == TTOT

    # wrapped int16 idx layout: [128, TTOT*P/16]
    idx_wrapped = np.stack(
        [np.tile(idx_flat[c].reshape(-1, 16).T, (8, 1)) for c in range(NCORES)]
    )
    dlb = np.stack(
        [dl_flat[c].reshape(TTOT, P).T.astype(ml_dtypes.bfloat16)
         for c in range(NCORES)]
    )  # [NCORES, 128, TTOT]

    # per-core degree ([128, NTC]) and x slices ([B, NTC*128, D])
    deg_r = np.ones((NCORES, P, NTC), dtype=np.float32)
    xs = np.zeros((NCORES, Bb, NTC * P, D), dtype=np.float32)
    xf = np.asarray(x, dtype=np.float32)
    for c in range(NCORES):
        for k in range(NTC):
            t = tids[c][k]
            n0 = t * P
            n1 = min(n0 + P, N)
            if n1 <= n0:
                continue
            deg_r[c, : n1 - n0, k] = deg[n0:n1]
            xs[c, :, k * P : k * P + (n1 - n0), :] = xf[:, n0:n1, :]

    # packed gather tables (both batches side by side), bf16
    xpack = np.concatenate([xf[0], xf[1]], axis=1).astype(ml_dtypes.bfloat16)
    xpa = np.ascontiguousarray(xpack[:SPLIT])
    xpb = np.ascontiguousarray(xpack[SPLIT:]) if N > SPLIT else None

    struct = dict(NTC=NTC, T0=T0, T1=T1, TTOT=TTOT, calls=calls,
                  NA=xpa.shape[0], NB=(xpb.shape[0] if xpb is not None else 0),
                  D=D, Bb=Bb)
    percore = dict(idx=idx_wrapped, dlb=dlb, deg=deg_r, xs=xs)
    shared = dict(xpa=xpa, xpb=xpb)
    return struct, percore, shared, tids, N


def _build(struct):
    import concourse.bacc as bacc
    import concourse.tile as tile
    from concourse import bass, mybir
    from concourse.masks import make_identity

    NTC, T0, T1, TTOT = struct["NTC"], struct["T0"], struct["T1"], struct["TTOT"]
    calls = struct["calls"]
    D = struct["D"]
    D2 = 2 * D
    TOTCOLS = TTOT * P // 16
    slots_max = max(t0 + t1 for t0, t1 in zip(T0, T1))
    f32, bf16, i16 = mybir.dt.float32, mybir.dt.bfloat16, mybir.dt.int16

    nc = bacc.Bacc("TRN2", target_bir_lowering=False, debug=False)
    d_xpa = nc.dram_tensor("xpa", [struct["NA"], D2], bf16, kind="ExternalInput")
    d_xpb = (nc.dram_tensor("xpb", [struct["NB"], D2], bf16, kind="ExternalInput")
             if struct["NB"] else None)
    d_xs = nc.dram_tensor("xs", [2, NTC * P, D], f32, kind="ExternalInput")
    d_idx = nc.dram_tensor("idx", [P, TOTCOLS], i16, kind="ExternalInput")
    d_dlb = nc.dram_tensor("dlb", [P, TTOT], bf16, kind="ExternalInput")
    d_deg = nc.dram_tensor("deg", [P, NTC], f32, kind="ExternalInput")
    d_w1 = nc.dram_tensor("w1", [D2, D2], f32, kind="ExternalInput")
    d_w2 = nc.dram_tensor("w2", [D2, D], f32, kind="ExternalInput")
    d_b1 = nc.dram_tensor("b1r", [P, 2], f32, kind="ExternalInput")
    d_b2 = nc.dram_tensor("b2r", [P, 1], f32, kind="ExternalInput")
    d_gx = nc.dram_tensor("gx", [P, 1], f32, kind="ExternalInput")
    d_gn = nc.dram_tensor("gn", [P, 1], f32, kind="ExternalInput")
    d_bx = nc.dram_tensor("bx", [P, 1], f32, kind="ExternalInput")
    d_bn = nc.dram_tensor("bn", [P, 1], f32, kind="ExternalInput")
    d_y = nc.dram_tensor("y", [2, NTC * P, D], f32, kind="ExternalOutput")

    with tile.TileContext(nc) as tc:
        with (
            tc.tile_pool(name="const", bufs=1) as cp,
            tc.tile_pool(name="gath", bufs=2) as gpool,
            tc.tile_pool(name="sel", bufs=2) as spool,
            tc.tile_pool(name="work", bufs=3) as wp,
            tc.tile_pool(name="ht", bufs=2) as hp,
            tc.tile_pool(name="nbps", bufs=2, space="PSUM") as nbps,
            tc.tile_pool(name="trps", bufs=2, space="PSUM") as trps,
            tc.tile_pool(name="mm1ps", bufs=2, space="PSUM") as mm1ps,
            tc.tile_pool(name="mm2ps", bufs=2, space="PSUM") as mm2ps,
        ):
            # ---- one-time constants ----
            idx_sb = cp.tile([P, TOTCOLS], i16)
            nc.sync.dma_start(idx_sb[:], d_idx.ap())
            dlb_sb = cp.tile([P, TTOT], bf16)
            nc.sync.dma_start(dlb_sb[:], d_dlb.ap())
            deg_sb = cp.tile([P, NTC], f32)
            nc.sync.dma_start(deg_sb[:], d_deg.ap())
            invd = cp.tile([P, NTC], f32)
            nc.vector.tensor_scalar_max(invd[:], deg_sb[:], 1.0)
            nc.vector.reciprocal(invd[:], invd[:])

            ident = cp.tile([P, P], f32)
            make_identity(nc, ident[:])
            iota8 = cp.tile([P, SCHUNK, P], bf16)
            nc.gpsimd.iota(iota8[:], pattern=[[0, SCHUNK], [1, P]], base=0,
                           channel_multiplier=0,
                           allow_small_or_imprecise_dtypes=True)
            eps_sb = cp.tile([P, 1], f32)
            nc.vector.memset(eps_sb[:], 1e-5)

            gx_sb = cp.tile([P, 1], f32); nc.sync.dma_start(gx_sb[:], d_gx.ap())
            gn_sb = cp.tile([P, 1], f32); nc.sync.dma_start(gn_sb[:], d_gn.ap())
            bx_sb = cp.tile([P, 1], f32); nc.sync.dma_start(bx_sb[:], d_bx.ap())
            bn_sb = cp.tile([P, 1], f32); nc.sync.dma_start(bn_sb[:], d_bn.ap())
            b1r_sb = cp.tile([P, 2], f32); nc.sync.dma_start(b1r_sb[:], d_b1.ap())
            b2r_sb = cp.tile([P, 1], f32); nc.sync.dma_start(b2r_sb[:], d_b2.ap())

            # W1 tiles [k-tile][j-tile], gamma-scaled copies, W2 tiles [k-tile]
            w1t = [[cp.tile([P, P], f32, name=f"w1t{kt}{jt}") for jt in range(2)]
                   for kt in range(2)]
            w1s = [[cp.tile([P, P], f32, name=f"w1s{kt}{jt}") for jt in range(2)]
                   for kt in range(2)]
            gam = [gx_sb, gn_sb]
            for kt in range(2):
                for jt in range(2):
                    nc.sync.dma_start(
                        w1t[kt][jt][:],
                        d_w1[kt * P : (kt + 1) * P, jt * P : (jt + 1) * P],
                    )
                    nc.vector.tensor_scalar_mul(
                        w1s[kt][jt][:], w1t[kt][jt][:], gam[kt][:]
                    )
            w2t = [cp.tile([P, P], f32, name=f"w2t{kt}") for kt in range(2)]
            for kt in range(2):
                nc.sync.dma_start(w2t[kt][:], d_w2[kt * P : (kt + 1) * P, :])

            # b1_eff = b1 + beta_cat @ W1  (per-partition layout [128, j-tile])
            bet = [bx_sb, bn_sb]
            b1b_ps = mm1ps.tile([P, 2], f32, space="PSUM", tag="m1")
            for jt in range(2):
                for kt in range(2):
                    nc.tensor.matmul(
                        b1b_ps[:, jt : jt + 1], lhsT=w1t[kt][jt][:],
                        rhs=bet[kt][:], start=(kt == 0), stop=(kt == 1),
                    )
            b1e_sb = cp.tile([P, 2], f32)
            nc.vector.tensor_add(b1e_sb[:], b1b_ps[:], b1r_sb[:])

            # per-position gather calls grouped by position
            calls_by_pos = {}
            for (k, srcg, so, nt, io) in calls:
                calls_by_pos.setdefault(k, []).append((srcg, so, nt, io))

            dl_off = np.concatenate([[0], np.cumsum(np.array(T0) + np.array(T1))])

            # ---- main loop over dst-tile positions ----
            for k in range(NTC):
                slots = T0[k] + T1[k]

                # neighbor aggregation -> nb_sb [128 dst, 2D]
                nb_sb = wp.tile([P, D2], f32, tag="nb")
                if slots == 0:
                    nc.vector.memset(nb_sb[:], 0.0)
                else:
                    g = gpool.tile([P, slots_max, D2], bf16, tag="g")
                    for (srcg, so, nt, io) in calls_by_pos.get(k, []):
                        src_t = d_xpa if srcg == "A" else d_xpb
                        nc.gpsimd.dma_gather(
                            g[:, so : so + nt, :], src_t.ap(),
                            idx_sb[:, io // 16 : (io + nt * P) // 16],
                            nt * P, nt * P, D2, single_packet=False,
                        )
                    S = spool.tile([P, slots_max, P], bf16, tag="S")
                    t0 = int(dl_off[k])
                    done = 0
                    while done < slots:
                        ns = min(slots - done, SCHUNK)
                        nc.vector.tensor_tensor(
                            out=S[:, done : done + ns, :],
                            in0=iota8[:, :ns, :],
                            in1=dlb_sb[:, t0 + done : t0 + done + ns].to_broadcast(
                                [P, ns, P]
                            ),
                            op=mybir.AluOpType.is_equal,
                        )
                        done += ns
                    nb_ps = nbps.tile([P, D2], f32, space="PSUM", tag="nbp")
                    for t in range(slots):
                        nc.tensor.matmul(
                            nb_ps[:], lhsT=S[:, t, :], rhs=g[:, t, :],
                            start=(t == 0), stop=(t == slots - 1),
                        )
                    nc.vector.tensor_scalar_mul(
                        nb_sb[:], nb_ps[:], invd[:, k : k + 1]
                    )

                # x slice [128, 2D] (b0 | b1)
                xs_sb = wp.tile([P, D2], f32, tag="xs")
                for b in range(2):
                    nc.sync.dma_start(
                        xs_sb[:, b * D : (b + 1) * D],
                        d_xs[b, k * P : (k + 1) * P, :],
                    )

                # LayerNorm stats: 4 instances (x b0, x b1, nb b0, nb b1)
                stx = wp.tile([P, 2, 6], f32, tag="stx")
                stn = wp.tile([P, 2, 6], f32, tag="stn")
                for b in range(2):
                    nc.vector.bn_stats(stx[:, b, :], xs_sb[:, b * D : (b + 1) * D])
                    nc.vector.bn_stats(stn[:, b, :], nb_sb[:, b * D : (b + 1) * D])
                mv4 = wp.tile([P, 4, 2], f32, tag="mv4")
                nc.vector.bn_aggr(mv4[:, 0, :], stx[:, 0:1, :])
                nc.vector.bn_aggr(mv4[:, 1, :], stx[:, 1:2, :])
                nc.vector.bn_aggr(mv4[:, 2, :], stn[:, 0:1, :])
                nc.vector.bn_aggr(mv4[:, 3, :], stn[:, 1:2, :])
                rs4 = wp.tile([P, 4], f32, tag="rs4")
                nc.scalar.activation(
                    rs4[:], mv4[:, :, 1], mybir.ActivationFunctionType.Sqrt,
                    bias=eps_sb[:], scale=1.0,
                )
                nc.vector.reciprocal(rs4[:], rs4[:])

                # normalized tiles (gamma/beta folded into W1/b1)
                hx = wp.tile([P, D2], f32, tag="hx")
                hn = wp.tile([P, D2], f32, tag="hn")
                for b in range(2):
                    nc.vector.tensor_scalar(
                        out=hx[:, b * D : (b + 1) * D],
                        in0=xs_sb[:, b * D : (b + 1) * D],
                        scalar1=mv4[:, b, 0:1], scalar2=rs4[:, b : b + 1],
                        op0=mybir.AluOpType.subtract, op1=mybir.AluOpType.mult,
                    )
                    nc.vector.tensor_scalar(
                        out=hn[:, b * D : (b + 1) * D],
                        in0=nb_sb[:, b * D : (b + 1) * D],
                        scalar1=mv4[:, 2 + b, 0:1], scalar2=rs4[:, 2 + b : 3 + b],
                        op0=mybir.AluOpType.subtract, op1=mybir.AluOpType.mult,
                    )

                # transpose to feature-major: hT[b][kt] [128 feat, 128 node]
                hT = [[hp.tile([P, P], f32, name=f"hT{b}{kt}", tag=f"hT{b}{kt}") for kt in range(2)]
                      for b in range(2)]
                for b in range(2):
                    for kt, srct in ((0, hx), (1, hn)):
                        tp = trps.tile([P, P], f32, space="PSUM", tag="tr")
                        nc.tensor.transpose(
                            tp[:], srct[:, b * D : (b + 1) * D], ident[:]
                        )
                        nc.scalar.copy(hT[b][kt][:], tp[:])

                # MLP
                y_sb = wp.tile([P, D2], f32, tag="y")
                for b in range(2):
                    gsb = [hp.tile([P, P], f32, name=f"g{b}{jt}", tag=f"g{b}{jt}") for jt in range(2)]
                    for jt in range(2):
                        m1 = mm1ps.tile([P, P], f32, space="PSUM", tag="m1")
                        for kt in range(2):
                            nc.tensor.matmul(
                                m1[:], lhsT=w1s[kt][jt][:], rhs=hT[b][kt][:],
                                start=(kt == 0), stop=(kt == 1),
                            )
                        nc.scalar.activation(
                            gsb[jt][:], m1[:], mybir.ActivationFunctionType.Gelu,
                            bias=b1e_sb[:, jt : jt + 1], scale=1.0,
                        )
                    m2 = mm2ps.tile([P, P], f32, space="PSUM", tag="m2")
                    for kt in range(2):
                        nc.tensor.matmul(
                            m2[:], lhsT=w2t[kt][:], rhs=gsb[kt][:],
                            start=(kt == 0), stop=(kt == 1),
                        )
                    o2 = wp.tile([P, P], f32, tag="o2")
                    nc.scalar.activation(
                        o2[:], m2[:], mybir.ActivationFunctionType.Identity,
                        bias=b2r_sb[:, 0:1], scale=1.0,
                    )
                    ot = trps.tile([P, P], f32, space="PSUM", tag="tr")
                    nc.tensor.transpose(ot[:], o2[:], ident[:])
                    nc.vector.tensor_add(
                        y_sb[:, b * D : (b + 1) * D], ot[:],
                        xs_sb[:, b * D : (b + 1) * D],
                    )
                for b in range(2):
                    nc.sync.dma_start(
                        d_y[b, k * P : (k + 1) * P, :],
                        y_sb[:, b * D : (b + 1) * D],
                    )
    nc.compile()
    return nc


def kernel(x, edge_src, edge_dst, degree, sn_g, sn_b, nn_g, nn_b, W1, b1, W2, b2):
    from concourse.bass_utils import run_bass_kernel_spmd

    x = np.asarray(x)
    Bb, N, D = x.shape
    assert Bb == 2 and D == P, (Bb, N, D)

    struct, percore, shared, tids, N = _prep(x, edge_src, edge_dst, degree)

    key = (struct["NTC"], tuple(struct["T0"]), tuple(struct["T1"]),
           struct["NA"], struct["NB"])
    if key not in _CACHE:
        _CACHE.clear()
        _CACHE[key] = _build(struct)
    nc = _CACHE[key]

    W1f = np.asarray(W1, dtype=np.float32)
    b1f = np.asarray(b1, dtype=np.float32).ravel()
    W2f = np.asarray(W2, dtype=np.float32)
    b2f = np.asarray(b2, dtype=np.float32).ravel()
    shared_map = dict(
        xpa=shared["xpa"],
        w1=np.ascontiguousarray(W1f),
        w2=np.ascontiguousarray(W2f),
        b1r=np.ascontiguousarray(b1f.reshape(2, P).T),
        b2r=np.ascontiguousarray(b2f.reshape(P, 1)),
        gx=np.asarray(sn_g, np.float32).reshape(P, 1),
        gn=np.asarray(nn_g, np.float32).reshape(P, 1),
        bx=np.asarray(sn_b, np.float32).reshape(P, 1),
        bn=np.asarray(nn_b, np.float32).reshape(P, 1),
    )
    if shared["xpb"] is not None:
        shared_map["xpb"] = shared["xpb"]

    in_maps = []
    for c in range(NCORES):
        m = dict(shared_map)
        m["xs"] = np.ascontiguousarray(percore["xs"][c])
        m["idx"] = np.ascontiguousarray(percore["idx"][c])
        m["dlb"] = np.ascontiguousarray(percore["dlb"][c])
        m["deg"] = np.ascontiguousarray(percore["deg"][c])
        in_maps.append(m)

    res = run_bass_kernel_spmd(nc, in_maps, core_ids=list(range(NCORES)))

    y = np.empty((Bb, N, D), dtype=np.float32)
    NTC = struct["NTC"]
    for c in range(NCORES):
        yc = res.results[c]["y"]
        for k in range(NTC):
            t = tids[c][k]
            n0 = t * P
            n1 = min(n0 + P, N)
            if n1 <= n0:
                continue
            y[:, n0:n1, :] = yc[:, k * P : k * P + (n1 - n0), :]
    return y


# revision 14
# speedup vs baseline: 1.2451x; 1.2451x over previous
"""Trainium2 Bass kernel for nn_MeshGraphBlock (GNN message-passing block).

Computes, for x:[B,N,D], edges (src,dst):[E], degree:[N]:
    neighbor = scatter_add(x[:, src, :] -> dst) / clip(degree, 1)
    h  = concat(LN(x; sn_g, sn_b), LN(neighbor; nn_g, nn_b))   # [B,N,2D]
    h  = gelu_erf(h @ W1 + b1)                                  # [B,N,2D]
    y  = x + h @ W2 + b2                                        # [B,N,D]

Strategy (8 NeuronCores, SPMD):
 - Destination-node tiles (128 nodes each) are assigned to cores via
   sorted round-robin so every core sees the same per-position edge-tile
   counts (the single compiled program is uniform; only data differs).
 - Host pre-sorts edges by dst, packs x (both batches side by side) as a
   bf16 [N,2D] table, and emits per-core gather indices (int16, split in
   two tables to stay under the 32767 index limit).
 - On device, edge messages are gathered with dma_gather (512B rows) and
   scatter-added into 128-dst PSUM accumulators via one-hot "selection
   matrix" matmuls (S[e,dst] built on DVE with iota==dst_local compares).
 - LayerNorm gamma/beta are folded into W1/b1 on device, LN itself uses
   bn_stats/bn_aggr; the MLP runs as PE matmuls with PE transposes to move
   between node-major and feature-major layouts.
"""

import math

import numpy as np
import ml_dtypes

P = 128
NCORES = 8
SPLIT = 32768          # int16 gather-index limit
MAX_TILES_PER_CALL = 8  # 1024 idxs per dma_gather (SWDGE ring limit)
SCHUNK = 8              # selection-matrix tiles built per DVE op
TUNE = dict(gpool=6, spool=6, wp=5, hp=3, group=4)

_CACHE = {}


def _prep(x, edge_src, edge_dst, degree):
    """Host-side sharding. Returns (structure, per-core inputs, assembly map)."""
    Bb, N, D = x.shape
    E = edge_src.shape[0]
    es = np.asarray(edge_src).astype(np.int64).ravel()
    ed = np.asarray(edge_dst).astype(np.int64).ravel()
    deg = np.asarray(degree).astype(np.float32).ravel()

    ntiles = math.ceil(N / P)
    ntiles_pad = math.ceil(ntiles / NCORES) * NCORES
    NTC = ntiles_pad // NCORES

    order = np.argsort(ed, kind="stable")
    ed_s = ed[order]
    es_s = es[order]
    bounds = np.searchsorted(ed_s, np.arange(ntiles_pad + 1) * P)

    counts = bounds[1:] - bounds[:-1]
    ranked = np.argsort(-counts, kind="stable")
    # tile ranked[i] -> core i % 8, position i // 8
    tids = [[0] * NTC for _ in range(NCORES)]
    for i, t in enumerate(ranked):
        tids[i % NCORES][i // NCORES] = int(t)

    # per (core, pos): split into G0 (src < SPLIT) and G1
    g0i, g1i, dli = {}, {}, {}
    for c in range(NCORES):
        for k in range(NTC):
            t = tids[c][k]
            a, b = bounds[t], bounds[t + 1]
            srcs = es_s[a:b]
            dloc = (ed_s[a:b] - t * P).astype(np.int64)
            m0 = srcs < SPLIT
            g0i[c, k] = srcs[m0].astype(np.int64)
            g1i[c, k] = (srcs[~m0] - SPLIT).astype(np.int64)
            dli[c, k] = (dloc[m0], dloc[~m0])

    T0 = [max(math.ceil(len(g0i[c, k]) / P) for c in range(NCORES)) for k in range(NTC)]
    T1 = [max(math.ceil(len(g1i[c, k]) / P) for c in range(NCORES)) for k in range(NTC)]

    # flat per-core index/dst-local streams in position order
    TTOT = sum(T0) + sum(T1)
    idx_flat = np.zeros((NCORES, TTOT * P), dtype=np.int16)
    dl_flat = np.full((NCORES, TTOT * P), -1.0, dtype=np.float32)
    calls = []  # (pos, 'A'|'B', slot_off, ntiles, idx_off) -- uniform across cores
    tile_off = 0
    for k in range(NTC):
        slot = 0
        for grp, T in ((0, T0[k]), (1, T1[k])):
            if T == 0:
                continue
            for c in range(NCORES):
                ii = g0i[c, k] if grp == 0 else g1i[c, k]
                dd = dli[c, k][grp]
                o = tile_off * P
                idx_flat[c, o : o + len(ii)] = ii.astype(np.int16)
                dl_flat[c, o : o + len(dd)] = dd.astype(np.float32)
            nt_done = 0
            while nt_done < T:
                nt = min(T - nt_done, MAX_TILES_PER_CALL)
                calls.append(
                    (k, "A" if grp == 0 else "B", slot + nt_done,
                     nt, (tile_off + nt_done) * P)
                )
                nt_done += nt
            tile_off += T
            slot += T
    assert tile_off == TTOT

    # wrapped int16 idx layout: [128, TTOT*P/16]
    idx_wrapped = np.stack(
        [np.tile(idx_flat[c].reshape(-1, 16).T, (8, 1)) for c in range(NCORES)]
    )
    dlb = np.stack(
        [dl_flat[c].reshape(TTOT, P).T.astype(ml_dtypes.bfloat16)
         for c in range(NCORES)]
    )  # [NCORES, 128, TTOT]

    # per-core degree ([128, NTC]) and x slices ([B, NTC*128, D])
    deg_r = np.ones((NCORES, P, NTC), dtype=np.float32)
    xs = np.zeros((NCORES, Bb, NTC * P, D), dtype=np.float32)
    xf = np.asarray(x, dtype=np.float32)
    for c in range(NCORES):
        for k in range(NTC):
            t = tids[c][k]
            n0 = t * P
            n1 = min(n0 + P, N)
            if n1 <= n0:
                continue
            deg_r[c, : n1 - n0, k] = deg[n0:n1]
            xs[c, :, k * P : k * P + (n1 - n0), :] = xf[:, n0:n1, :]

    # packed gather tables (both batches side by side), bf16
    xpack = np.concatenate([xf[0], xf[1]], axis=1).astype(ml_dtypes.bfloat16)
    xpa = np.ascontiguousarray(xpack[:SPLIT])
    xpb = np.ascontiguousarray(xpack[SPLIT:]) if N > SPLIT else None

    struct = dict(NTC=NTC, T0=T0, T1=T1, TTOT=TTOT, calls=calls,
                  NA=xpa.shape[0], NB=(xpb.shape[0] if xpb is not None else 0),
                  D=D, Bb=Bb)
    percore = dict(idx=idx_wrapped, dlb=dlb, deg=deg_r, xs=xs)
    shared = dict(xpa=xpa, xpb=xpb)
    return struct, percore, shared, tids, N


def _build(struct, ablate=frozenset()):
    import concourse.bacc as bacc
    import concourse.tile as tile
    from concourse import bass, mybir
    from concourse.masks import make_identity

    NTC, T0, T1, TTOT = struct["NTC"], struct["T0"], struct["T1"], struct["TTOT"]
    calls = struct["calls"]
    D = struct["D"]
    D2 = 2 * D
    TOTCOLS = TTOT * P // 16
    slots_max = max(t0 + t1 for t0, t1 in zip(T0, T1))
    f32, bf16, i16 = mybir.dt.float32, mybir.dt.bfloat16, mybir.dt.int16

    nc = bacc.Bacc("TRN2", target_bir_lowering=False, debug=False)
    d_xpa = nc.dram_tensor("xpa", [struct["NA"], D2], bf16, kind="ExternalInput")
    d_xpb = (nc.dram_tensor("xpb", [struct["NB"], D2], bf16, kind="ExternalInput")
             if struct["NB"] else None)
    d_xs = nc.dram_tensor("xs", [2, NTC * P, D], f32, kind="ExternalInput")
    d_idx = nc.dram_tensor("idx", [P, TOTCOLS], i16, kind="ExternalInput")
    d_dlb = nc.dram_tensor("dlb", [P, TTOT], bf16, kind="ExternalInput")
    d_deg = nc.dram_tensor("deg", [P, NTC], f32, kind="ExternalInput")
    d_w1 = nc.dram_tensor("w1", [D2, D2], f32, kind="ExternalInput")
    d_w2 = nc.dram_tensor("w2", [D2, D], f32, kind="ExternalInput")
    d_b1 = nc.dram_tensor("b1r", [P, 2], f32, kind="ExternalInput")
    d_b2 = nc.dram_tensor("b2r", [1, P], f32, kind="ExternalInput")
    d_gx = nc.dram_tensor("gx", [P, 1], f32, kind="ExternalInput")
    d_gn = nc.dram_tensor("gn", [P, 1], f32, kind="ExternalInput")
    d_bx = nc.dram_tensor("bx", [P, 1], f32, kind="ExternalInput")
    d_bn = nc.dram_tensor("bn", [P, 1], f32, kind="ExternalInput")
    d_y = nc.dram_tensor("y", [2, NTC * P, D], f32, kind="ExternalOutput")

    with tile.TileContext(nc) as tc:
        with (
            tc.tile_pool(name="const", bufs=1) as cp,
            tc.tile_pool(name="gath", bufs=TUNE["gpool"]) as gpool,
            tc.tile_pool(name="sel", bufs=TUNE["spool"]) as spool,
            tc.tile_pool(name="work", bufs=TUNE["wp"]) as wp,
            tc.tile_pool(name="ht", bufs=TUNE["hp"]) as hp,
            tc.tile_pool(name="nbps", bufs=2, space="PSUM") as nbps,
            tc.tile_pool(name="trps", bufs=2, space="PSUM") as trps,
            tc.tile_pool(name="mm1ps", bufs=2, space="PSUM") as mm1ps,
            tc.tile_pool(name="mm2ps", bufs=2, space="PSUM") as mm2ps,
        ):
            # ---- one-time constants ----
            idx_sb = cp.tile([P, TOTCOLS], i16)
            nc.sync.dma_start(idx_sb[:], d_idx.ap())
            dlb_sb = cp.tile([P, TTOT], bf16)
            nc.sync.dma_start(dlb_sb[:], d_dlb.ap())
            deg_sb = cp.tile([P, NTC], f32)
            nc.sync.dma_start(deg_sb[:], d_deg.ap())
            invd = cp.tile([P, NTC], f32)
            nc.vector.tensor_scalar_max(invd[:], deg_sb[:], 1.0)
            nc.vector.reciprocal(invd[:], invd[:])

            ident = cp.tile([P, P], f32)
            make_identity(nc, ident[:])
            iota8 = cp.tile([P, SCHUNK, P], bf16)
            nc.gpsimd.iota(iota8[:], pattern=[[0, SCHUNK], [1, P]], base=0,
                           channel_multiplier=0,
                           allow_small_or_imprecise_dtypes=True)
            eps_sb = cp.tile([P, 1], f32)
            nc.vector.memset(eps_sb[:], 1e-5)

            gx_sb = cp.tile([P, 1], f32); nc.sync.dma_start(gx_sb[:], d_gx.ap())
            gn_sb = cp.tile([P, 1], f32); nc.sync.dma_start(gn_sb[:], d_gn.ap())
            bx_sb = cp.tile([P, 1], f32); nc.sync.dma_start(bx_sb[:], d_bx.ap())
            bn_sb = cp.tile([P, 1], f32); nc.sync.dma_start(bn_sb[:], d_bn.ap())
            b1r_sb = cp.tile([P, 2], f32); nc.sync.dma_start(b1r_sb[:], d_b1.ap())
            b2r_sb = cp.tile([1, P], f32); nc.sync.dma_start(b2r_sb[:], d_b2.ap())
            ones1 = cp.tile([1, P], f32)
            nc.vector.memset(ones1[:], 1.0)

            # W1 tiles [k-tile][j-tile], gamma-scaled copies, W2 tiles [k-tile]
            w1t = [[cp.tile([P, P], f32, name=f"w1t{kt}{jt}") for jt in range(2)]
                   for kt in range(2)]
            w1s = [[cp.tile([P, P], f32, name=f"w1s{kt}{jt}") for jt in range(2)]
                   for kt in range(2)]
            gam = [gx_sb, gn_sb]
            for kt in range(2):
                for jt in range(2):
                    nc.sync.dma_start(
                        w1t[kt][jt][:],
                        d_w1[kt * P : (kt + 1) * P, jt * P : (jt + 1) * P],
                    )
                    nc.vector.tensor_scalar_mul(
                        w1s[kt][jt][:], w1t[kt][jt][:], gam[kt][:]
                    )
            w2t = [cp.tile([P, P], f32, name=f"w2t{kt}") for kt in range(2)]
            for kt in range(2):
                nc.sync.dma_start(w2t[kt][:], d_w2[kt * P : (kt + 1) * P, :])

            # b1_eff = b1 + beta_cat @ W1  (per-partition layout [128, j-tile])
            bet = [bx_sb, bn_sb]
            b1b_ps = mm1ps.tile([P, 2], f32, space="PSUM", tag="m1")
            for jt in range(2):
                for kt in range(2):
                    nc.tensor.matmul(
                        b1b_ps[:, jt : jt + 1], lhsT=w1t[kt][jt][:],
                        rhs=bet[kt][:], start=(kt == 0), stop=(kt == 1),
                    )
            b1e_sb = cp.tile([P, 2], f32)
            nc.vector.tensor_add(b1e_sb[:], b1b_ps[:], b1r_sb[:])

            # per-position gather calls grouped by position
            calls_by_pos = {}
            for (k, srcg, so, nt, io) in calls:
                calls_by_pos.setdefault(k, []).append((srcg, so, nt, io))

            dl_off = np.concatenate([[0], np.cumsum(np.array(T0) + np.array(T1))])

            # ---- main loop: groups of GROUP positions ----
            GROUP = TUNE["group"]
            for k0 in range(0, NTC, GROUP):
                gs = min(GROUP, NTC - k0)
                nb_t, xs_t = {}, {}
                mvg = wp.tile([P, 4 * gs, 2], f32, tag="mvg", bufs=2,
                              name=f"mvg{k0}")
                # phase A: aggregate neighbors + stats
                for gi in range(gs):
                    k = k0 + gi
                    slots = T0[k] + T1[k]
                    nb_sb = wp.tile([P, D2], f32, tag="nb", bufs=GROUP + 2,
                                    name=f"nb{k}")
                    nb_t[k] = nb_sb
                    if slots == 0 or "segsum" in ablate:
                        nc.vector.memset(nb_sb[:], 0.0)
                    else:
                        g = gpool.tile([P, slots_max, D2], bf16, tag="g",
                                       name=f"g{k}")
                        if "gather" in ablate:
                            nc.vector.memset(g[:, 0, :], 0.0)
                        else:
                            for (srcg, so, nt, io) in calls_by_pos.get(k, []):
                                src_t = d_xpa if srcg == "A" else d_xpb
                                nc.gpsimd.dma_gather(
                                    g[:, so : so + nt, :], src_t.ap(),
                                    idx_sb[:, io // 16 : (io + nt * P) // 16],
                                    nt * P, nt * P, D2, single_packet=False,
                                )
                        S = spool.tile([P, slots_max, P], bf16, tag="S",
                                       name=f"S{k}")
                        if "sgen" in ablate:
                            nc.vector.memset(S[:, 0, :], 0.0)
                        else:
                            t0 = int(dl_off[k])
                            done = 0
                            while done < slots:
                                ns = min(slots - done, SCHUNK)
                                nc.vector.tensor_tensor(
                                    out=S[:, done : done + ns, :],
                                    in0=iota8[:, :ns, :],
                                    in1=dlb_sb[:, t0 + done : t0 + done + ns]
                                    .to_broadcast([P, ns, P]),
                                    op=mybir.AluOpType.is_equal,
                                )
                                done += ns
                        nb_ps = nbps.tile([P, D2], f32, space="PSUM", tag="nbp",
                                          name=f"nbp{k}")
                        nmm = 1 if "segmm" in ablate else slots
                        for t in range(nmm):
                            nc.tensor.matmul(
                                nb_ps[:], lhsT=S[:, t, :], rhs=g[:, t, :],
                                start=(t == 0), stop=(t == nmm - 1),
                            )
                        # evacuate with 1/deg scaling on ScalarE
                        nc.scalar.activation(
                            nb_sb[:], nb_ps[:],
                            mybir.ActivationFunctionType.Copy,
                            scale=invd[:, k : k + 1],
                        )

                    xs_sb = wp.tile([P, D2], f32, tag="xs", bufs=GROUP + 2,
                                    name=f"xs{k}")
                    xs_t[k] = xs_sb
                    for b in range(2):
                        nc.sync.dma_start(
                            xs_sb[:, b * D : (b + 1) * D],
                            d_xs[b, k * P : (k + 1) * P, :],
                        )
                    stx = wp.tile([P, 2, 6], f32, tag="stx", name=f"stx{k}")
                    stn = wp.tile([P, 2, 6], f32, tag="stn", name=f"stn{k}")
                    for b in range(2):
                        nc.vector.bn_stats(stx[:, b, :], xs_sb[:, b * D : (b + 1) * D])
                        nc.vector.bn_stats(stn[:, b, :], nb_sb[:, b * D : (b + 1) * D])
                    nc.vector.bn_aggr(mvg[:, 4 * gi + 0, :], stx[:, 0:1, :])
                    nc.vector.bn_aggr(mvg[:, 4 * gi + 1, :], stx[:, 1:2, :])
                    nc.vector.bn_aggr(mvg[:, 4 * gi + 2, :], stn[:, 0:1, :])
                    nc.vector.bn_aggr(mvg[:, 4 * gi + 3, :], stn[:, 1:2, :])

                # group-level rstd: one Sqrt (no act-table thrash) + reciprocal
                rsg = wp.tile([P, 4 * gs], f32, tag="rsg", bufs=2, name=f"rsg{k0}")
                nc.scalar.activation(
                    rsg[:], mvg[:, :, 1], mybir.ActivationFunctionType.Sqrt,
                    bias=eps_sb[:], scale=1.0,
                )
                nc.vector.reciprocal(rsg[:], rsg[:])

                # phase B: normalize + MLP + residual
                for gi in range(gs):
                    k = k0 + gi
                    nb_sb, xs_sb = nb_t[k], xs_t[k]
                    hx = wp.tile([P, D2], f32, tag="hx", name=f"hx{k}")
                    hn = wp.tile([P, D2], f32, tag="hn", name=f"hn{k}")
                    for b in range(2):
                        nc.vector.tensor_scalar(
                            out=hx[:, b * D : (b + 1) * D],
                            in0=xs_sb[:, b * D : (b + 1) * D],
                            scalar1=mvg[:, 4 * gi + b, 0:1],
                            scalar2=rsg[:, 4 * gi + b : 4 * gi + b + 1],
                            op0=mybir.AluOpType.subtract,
                            op1=mybir.AluOpType.mult,
                        )
                        nc.vector.tensor_scalar(
                            out=hn[:, b * D : (b + 1) * D],
                            in0=nb_sb[:, b * D : (b + 1) * D],
                            scalar1=mvg[:, 4 * gi + 2 + b, 0:1],
                            scalar2=rsg[:, 4 * gi + 2 + b : 4 * gi + 3 + b],
                            op0=mybir.AluOpType.subtract,
                            op1=mybir.AluOpType.mult,
                        )

                    # feature-major h via PE transposes
                    hT = [[hp.tile([P, P], f32, name=f"hT{b}{kt}_{k}",
                                   tag=f"hT{b}{kt}") for kt in range(2)]
                          for b in range(2)]
                    for b in range(2):
                        for kt, srct in ((0, hx), (1, hn)):
                            tp = trps.tile([P, P], f32, space="PSUM", tag="tr",
                                           name=f"tr{b}{kt}_{k}")
                            nc.tensor.transpose(
                                tp[:], srct[:, b * D : (b + 1) * D], ident[:]
                            )
                            nc.scalar.copy(hT[b][kt][:], tp[:])

                    y_sb = wp.tile([P, D2], f32, tag="y", name=f"y{k}")
                    for b in range(2):
                        gsb = [hp.tile([P, P], f32, name=f"g{b}{jt}_{k}",
                                       tag=f"g{b}{jt}") for jt in range(2)]
                        for jt in range(2):
                            m1 = mm1ps.tile([P, P], f32, space="PSUM", tag="m1",
                                            name=f"m1_{b}{jt}_{k}")
                            for kt in range(2):
                                nc.tensor.matmul(
                                    m1[:], lhsT=w1s[kt][jt][:], rhs=hT[b][kt][:],
                                    start=(kt == 0), stop=(kt == 1),
                                )
                            nc.scalar.activation(
                                gsb[jt][:], m1[:],
                                mybir.ActivationFunctionType.Gelu,
                                bias=b1e_sb[:, jt : jt + 1], scale=1.0,
                            )
                        # y_psum = g^T @ W2 + I @ x + ones x b2  (node-major out)
                        m2 = mm2ps.tile([P, P], f32, space="PSUM", tag="m2",
                                        name=f"m2_{b}_{k}")
                        for kt in range(2):
                            nc.tensor.matmul(
                                m2[:], lhsT=gsb[kt][:], rhs=w2t[kt][:],
                                start=(kt == 0), stop=False,
                            )
                        nc.tensor.matmul(
                            m2[:], lhsT=ident[:],
                            rhs=xs_sb[:, b * D : (b + 1) * D],
                            start=False, stop=False,
                        )
                        nc.tensor.matmul(
                            m2[:], lhsT=ones1[:], rhs=b2r_sb[:],
                            start=False, stop=True,
                        )
                        nc.scalar.copy(y_sb[:, b * D : (b + 1) * D], m2[:])
                    for b in range(2):
                        nc.sync.dma_start(
                            d_y[b, k * P : (k + 1) * P, :],
                            y_sb[:, b * D : (b + 1) * D],
                        )
    nc.compile()
    return nc


def kernel(x, edge_src, edge_dst, degree, sn_g, sn_b, nn_g, nn_b, W1, b1, W2, b2):
    from concourse.bass_utils import run_bass_kernel_spmd

    x = np.asarray(x)
    Bb, N, D = x.shape
    assert Bb == 2 and D == P, (Bb, N, D)

    struct, percore, shared, tids, N = _prep(x, edge_src, edge_dst, degree)

    key = (struct["NTC"], tuple(struct["T0"]), tuple(struct["T1"]),
           struct["NA"], struct["NB"])
    if key not in _CACHE:
        _CACHE.clear()
        _CACHE[key] = _build(struct)
    nc = _CACHE[key]

    W1f = np.asarray(W1, dtype=np.float32)
    b1f = np.asarray(b1, dtype=np.float32).ravel()
    W2f = np.asarray(W2, dtype=np.float32)
    b2f = np.asarray(b2, dtype=np.float32).ravel()
    shared_map = dict(
        xpa=shared["xpa"],
        w1=np.ascontiguousarray(W1f),
        w2=np.ascontiguousarray(W2f),
        b1r=np.ascontiguousarray(b1f.reshape(2, P).T),
        b2r=np.ascontiguousarray(b2f.reshape(1, P)),
        gx=np.asarray(sn_g, np.float32).reshape(P, 1),
        gn=np.asarray(nn_g, np.float32).reshape(P, 1),
        bx=np.asarray(sn_b, np.float32).reshape(P, 1),
        bn=np.asarray(nn_b, np.float32).reshape(P, 1),
    )
    if shared["xpb"] is not None:
        shared_map["xpb"] = shared["xpb"]

    in_maps = []
    for c in range(NCORES):
        m = dict(shared_map)
        m["xs"] = np.ascontiguousarray(percore["xs"][c])
        m["idx"] = np.ascontiguousarray(percore["idx"][c])
        m["dlb"] = np.ascontiguousarray(percore["dlb"][c])
        m["deg"] = np.ascontiguousarray(percore["deg"][c])
        in_maps.append(m)

    res = run_bass_kernel_spmd(nc, in_maps, core_ids=list(range(NCORES)))

    y = np.empty((Bb, N, D), dtype=np.float32)
    NTC = struct["NTC"]
    for c in range(NCORES):
        yc = res.results[c]["y"]
        for k in range(NTC):
            t = tids[c][k]
            n0 = t * P
            n1 = min(n0 + P, N)
            if n1 <= n0:
                continue
            y[:, n0:n1, :] = yc[:, k * P : k * P + (n1 - n0), :]
    return y


# revision 21
# speedup vs baseline: 1.5403x; 1.2370x over previous
"""Trainium2 Bass kernel for nn_MeshGraphBlock (GNN message-passing block).

Computes, for x:[B,N,D], edges (src,dst):[E], degree:[N]:
    neighbor = scatter_add(x[:, src, :] -> dst) / clip(degree, 1)
    h  = concat(LN(x; sn_g, sn_b), LN(neighbor; nn_g, nn_b))   # [B,N,2D]
    h  = gelu_erf(h @ W1 + b1)                                  # [B,N,2D]
    y  = x + h @ W2 + b2                                        # [B,N,D]

Strategy (8 NeuronCores, SPMD):
 - Destination-node tiles (128 nodes each) are assigned to cores via
   sorted round-robin so every core sees the same per-position edge-tile
   counts (the single compiled program is uniform; only data differs).
 - Host pre-sorts edges by dst, packs x (both batches side by side) as a
   bf16 [N,2D] table, and emits per-core gather indices (int16, split in
   two tables to stay under the 32767 index limit).
 - On device, edge messages are gathered with dma_gather (512B rows) and
   scatter-added into 128-dst PSUM accumulators via one-hot "selection
   matrix" matmuls (S[e,dst] built on DVE with iota==dst_local compares).
 - LayerNorm gamma/beta are folded into W1/b1 on device, LN itself uses
   bn_stats/bn_aggr; the MLP runs as PE matmuls with PE transposes to move
   between node-major and feature-major layouts.
"""

import math

import numpy as np
import ml_dtypes

P = 128
NCORES = 8
SPLIT = 32768          # int16 gather-index limit
MAX_TILES_PER_CALL = 8  # 1024 idxs per dma_gather (SWDGE ring limit)
SCHUNK = 8              # selection-matrix tiles built per DVE op
TUNE = dict(gpool=6, spool=6, wp=5, hp=3, group=3)

_CACHE = {}


def _prep(x, edge_src, edge_dst, degree):
    """Host-side sharding. Returns (structure, per-core inputs, assembly map)."""
    Bb, N, D = x.shape
    E = edge_src.shape[0]
    es = np.asarray(edge_src).astype(np.int64).ravel()
    ed = np.asarray(edge_dst).astype(np.int64).ravel()
    deg = np.asarray(degree).astype(np.float32).ravel()

    ntiles = math.ceil(N / P)
    ntiles_pad = math.ceil(ntiles / NCORES) * NCORES
    NTC = ntiles_pad // NCORES

    order = np.argsort(ed, kind="stable")
    ed_s = ed[order]
    es_s = es[order]
    bounds = np.searchsorted(ed_s, np.arange(ntiles_pad + 1) * P)

    counts = bounds[1:] - bounds[:-1]
    ranked = np.argsort(-counts, kind="stable")
    # tile ranked[i] -> core i % 8, position i // 8
    tids = [[0] * NTC for _ in range(NCORES)]
    for i, t in enumerate(ranked):
        tids[i % NCORES][i // NCORES] = int(t)

    # per (core, pos): split into G0 (src < SPLIT) and G1
    g0i, g1i, dli = {}, {}, {}
    for c in range(NCORES):
        for k in range(NTC):
            t = tids[c][k]
            a, b = bounds[t], bounds[t + 1]
            srcs = es_s[a:b]
            dloc = (ed_s[a:b] - t * P).astype(np.int64)
            m0 = srcs < SPLIT
            g0i[c, k] = srcs[m0].astype(np.int64)
            g1i[c, k] = (srcs[~m0] - SPLIT).astype(np.int64)
            dli[c, k] = (dloc[m0], dloc[~m0])

    T0 = [max(math.ceil(len(g0i[c, k]) / P) for c in range(NCORES)) for k in range(NTC)]
    T1 = [max(math.ceil(len(g1i[c, k]) / P) for c in range(NCORES)) for k in range(NTC)]

    # flat per-core index/dst-local streams in position order
    TTOT = sum(T0) + sum(T1)
    idx_flat = np.zeros((NCORES, TTOT * P), dtype=np.int16)
    dl_flat = np.full((NCORES, TTOT * P), -1.0, dtype=np.float32)
    calls = []  # (pos, 'A'|'B', slot_off, ntiles, idx_off) -- uniform across cores
    tile_off = 0
    for k in range(NTC):
        slot = 0
        for grp, T in ((0, T0[k]), (1, T1[k])):
            if T == 0:
                continue
            for c in range(NCORES):
                ii = g0i[c, k] if grp == 0 else g1i[c, k]
                dd = dli[c, k][grp]
                o = tile_off * P
                idx_flat[c, o : o + len(ii)] = ii.astype(np.int16)
                dl_flat[c, o : o + len(dd)] = dd.astype(np.float32)
            nt_done = 0
            while nt_done < T:
                nt = min(T - nt_done, MAX_TILES_PER_CALL)
                calls.append(
                    (k, "A" if grp == 0 else "B", slot + nt_done,
                     nt, (tile_off + nt_done) * P)
                )
                nt_done += nt
            tile_off += T
            slot += T
    assert tile_off == TTOT

    # wrapped int16 idx layout: [128, TTOT*P/16]
    idx_wrapped = np.stack(
        [np.tile(idx_flat[c].reshape(-1, 16).T, (8, 1)) for c in range(NCORES)]
    )
    dlb = np.stack(
        [np.ascontiguousarray(dl_flat[c].reshape(TTOT, P).T)
         for c in range(NCORES)]
    )  # [NCORES, 128, TTOT] float32

    # per-core degree ([128, NTC]) and x slices ([B, NTC*128, D])
    deg_r = np.ones((NCORES, P, NTC), dtype=np.float32)
    xs = np.zeros((NCORES, NTC * P, 2 * D), dtype=np.float32)
    xf = np.asarray(x, dtype=np.float32)
    for c in range(NCORES):
        for k in range(NTC):
            t = tids[c][k]
            n0 = t * P
            n1 = min(n0 + P, N)
            if n1 <= n0:
                continue
            deg_r[c, : n1 - n0, k] = deg[n0:n1]
            xs[c, k * P : k * P + (n1 - n0), :D] = xf[0, n0:n1, :]
            xs[c, k * P : k * P + (n1 - n0), D:] = xf[1, n0:n1, :]

    # packed gather tables (both batches side by side), bf16
    xpack = np.concatenate([xf[0], xf[1]], axis=1).astype(ml_dtypes.bfloat16)
    xpa = np.ascontiguousarray(xpack[:SPLIT])
    xpb = np.ascontiguousarray(xpack[SPLIT:]) if N > SPLIT else None

    struct = dict(NTC=NTC, T0=T0, T1=T1, TTOT=TTOT, calls=calls,
                  NA=xpa.shape[0], NB=(xpb.shape[0] if xpb is not None else 0),
                  D=D, Bb=Bb)
    percore = dict(idx=idx_wrapped, dlb=dlb, deg=deg_r, xs=xs)
    shared = dict(xpa=xpa, xpb=xpb)
    return struct, percore, shared, tids, N


def _build(struct, ablate=frozenset()):
    import concourse.bacc as bacc
    import concourse.tile as tile
    from concourse import bass, mybir
    from concourse.masks import make_identity

    NTC, T0, T1, TTOT = struct["NTC"], struct["T0"], struct["T1"], struct["TTOT"]
    calls = struct["calls"]
    D = struct["D"]
    D2 = 2 * D
    TOTCOLS = TTOT * P // 16
    slots_max = max(t0 + t1 for t0, t1 in zip(T0, T1))
    f32, bf16, i16 = mybir.dt.float32, mybir.dt.bfloat16, mybir.dt.int16
    f32r = mybir.dt.float32r

    nc = bacc.Bacc("TRN2", target_bir_lowering=False, debug=False)
    d_xpa = nc.dram_tensor("xpa", [struct["NA"], D2], bf16, kind="ExternalInput")
    d_xpb = (nc.dram_tensor("xpb", [struct["NB"], D2], bf16, kind="ExternalInput")
             if struct["NB"] else None)
    d_xs = nc.dram_tensor("xs", [NTC * P, D2], f32, kind="ExternalInput")
    d_idx = nc.dram_tensor("idx", [P, TOTCOLS], i16, kind="ExternalInput")
    d_dlb = nc.dram_tensor("dlb", [P, TTOT], f32, kind="ExternalInput")
    d_deg = nc.dram_tensor("deg", [P, NTC], f32, kind="ExternalInput")
    d_w1 = nc.dram_tensor("w1", [D2, D2], f32, kind="ExternalInput")
    d_w2 = nc.dram_tensor("w2", [D2, D], f32, kind="ExternalInput")
    d_b1 = nc.dram_tensor("b1r", [P, 2], f32, kind="ExternalInput")
    d_b2 = nc.dram_tensor("b2r", [1, P], f32, kind="ExternalInput")
    d_gx = nc.dram_tensor("gx", [P, 1], f32, kind="ExternalInput")
    d_gn = nc.dram_tensor("gn", [P, 1], f32, kind="ExternalInput")
    d_bx = nc.dram_tensor("bx", [P, 1], f32, kind="ExternalInput")
    d_bn = nc.dram_tensor("bn", [P, 1], f32, kind="ExternalInput")
    d_y = nc.dram_tensor("y", [NTC * P, D2], f32, kind="ExternalOutput")

    with tile.TileContext(nc) as tc:
        with (
            tc.tile_pool(name="const", bufs=1) as cp,
            tc.tile_pool(name="gath", bufs=TUNE["gpool"]) as gpool,
            tc.tile_pool(name="sel", bufs=TUNE["spool"]) as spool,
            tc.tile_pool(name="work", bufs=TUNE["wp"]) as wp,
            tc.tile_pool(name="ht", bufs=TUNE["hp"]) as hp,
            tc.tile_pool(name="nbps", bufs=2, space="PSUM") as nbps,
            tc.tile_pool(name="trps", bufs=2, space="PSUM") as trps,
            tc.tile_pool(name="mm1ps", bufs=2, space="PSUM") as mm1ps,
            tc.tile_pool(name="mm2ps", bufs=2, space="PSUM") as mm2ps,
        ):
            # ---- one-time constants ----
            idx_sb = cp.tile([P, TOTCOLS], i16)
            nc.sync.dma_start(idx_sb[:], d_idx.ap())
            dlb_sb = cp.tile([P, TTOT], f32)
            nc.sync.dma_start(dlb_sb[:], d_dlb.ap())
            deg_sb = cp.tile([P, NTC], f32)
            nc.sync.dma_start(deg_sb[:], d_deg.ap())
            invd = cp.tile([P, NTC], f32)
            nc.vector.tensor_scalar_max(invd[:], deg_sb[:], 1.0)
            nc.vector.reciprocal(invd[:], invd[:])

            ident = cp.tile([P, P], f32)
            make_identity(nc, ident[:])
            iota1 = cp.tile([P, P], bf16)
            nc.gpsimd.iota(iota1[:], pattern=[[1, P]], base=0,
                           channel_multiplier=0,
                           allow_small_or_imprecise_dtypes=True)
            eps_sb = cp.tile([P, 1], f32)
            nc.vector.memset(eps_sb[:], 1e-5)

            gx_sb = cp.tile([P, 1], f32); nc.sync.dma_start(gx_sb[:], d_gx.ap())
            gn_sb = cp.tile([P, 1], f32); nc.sync.dma_start(gn_sb[:], d_gn.ap())
            bx_sb = cp.tile([P, 1], f32); nc.sync.dma_start(bx_sb[:], d_bx.ap())
            bn_sb = cp.tile([P, 1], f32); nc.sync.dma_start(bn_sb[:], d_bn.ap())
            b1r_sb = cp.tile([P, 2], f32); nc.sync.dma_start(b1r_sb[:], d_b1.ap())
            b2r_sb = cp.tile([1, P], f32); nc.sync.dma_start(b2r_sb[:], d_b2.ap())
            ones1 = cp.tile([1, P], f32)
            nc.vector.memset(ones1[:], 1.0)

            # W1 tiles [k-tile][j-tile], gamma-scaled copies, W2 tiles [k-tile]
            w1t = [[cp.tile([P, P], f32, name=f"w1t{kt}{jt}") for jt in range(2)]
                   for kt in range(2)]
            w1s = [[cp.tile([P, P], bf16, name=f"w1s{kt}{jt}") for jt in range(2)]
                   for kt in range(2)]
            gam = [gx_sb, gn_sb]
            for kt in range(2):
                for jt in range(2):
                    nc.sync.dma_start(
                        w1t[kt][jt][:],
                        d_w1[kt * P : (kt + 1) * P, jt * P : (jt + 1) * P],
                    )
                    nc.vector.tensor_scalar_mul(
                        w1s[kt][jt][:], w1t[kt][jt][:], gam[kt][:]
                    )
            w2t = [cp.tile([P, P], f32, name=f"w2t{kt}") for kt in range(2)]
            for kt in range(2):
                nc.sync.dma_start(w2t[kt][:], d_w2[kt * P : (kt + 1) * P, :])

            # b1_eff = b1 + beta_cat @ W1  (per-partition layout [128, j-tile])
            bet = [bx_sb, bn_sb]
            b1b_ps = mm1ps.tile([P, 2], f32, space="PSUM", tag="m1")
            for jt in range(2):
                for kt in range(2):
                    nc.tensor.matmul(
                        b1b_ps[:, jt : jt + 1], lhsT=w1t[kt][jt][:],
                        rhs=bet[kt][:], start=(kt == 0), stop=(kt == 1),
                    )
            b1e_sb = cp.tile([P, 2], f32)
            nc.vector.tensor_add(b1e_sb[:], b1b_ps[:], b1r_sb[:])

            # per-position gather calls grouped by position
            calls_by_pos = {}
            for (k, srcg, so, nt, io) in calls:
                calls_by_pos.setdefault(k, []).append((srcg, so, nt, io))

            dl_off = np.concatenate([[0], np.cumsum(np.array(T0) + np.array(T1))])

            # ---- main loop: groups of GROUP positions ----
            GROUP = TUNE["group"]
            for k0 in range(0, NTC, GROUP):
                gs = min(GROUP, NTC - k0)
                nb_t, xs_t = {}, {}
                mvg = wp.tile([P, 4 * gs, 2], f32, tag="mvg", bufs=2,
                              name=f"mvg{k0}")
                # phase A: aggregate neighbors + stats
                for gi in range(gs):
                    k = k0 + gi
                    slots = T0[k] + T1[k]
                    nb_sb = wp.tile([P, D2], f32, tag="nb", bufs=GROUP + 2,
                                    name=f"nb{k}")
                    nb_t[k] = nb_sb
                    if slots == 0 or "segsum" in ablate:
                        nc.vector.memset(nb_sb[:], 0.0)
                    else:
                        g = gpool.tile([P, slots_max, D2], bf16, tag="g",
                                       name=f"g{k}")
                        if "gather" in ablate:
                            nc.vector.memset(g[:, 0, :], 0.0)
                        else:
                            for (srcg, so, nt, io) in calls_by_pos.get(k, []):
                                src_t = d_xpa if srcg == "A" else d_xpb
                                nc.gpsimd.dma_gather(
                                    g[:, so : so + nt, :], src_t.ap(),
                                    idx_sb[:, io // 16 : (io + nt * P) // 16],
                                    nt * P, nt * P, D2, single_packet=False,
                                )
                        S = spool.tile([P, slots_max, P], bf16, tag="S",
                                       name=f"S{k}")
                        if "sgen" in ablate:
                            nc.vector.memset(S[:, 0, :], 0.0)
                        else:
                            t0 = int(dl_off[k])
                            for t in range(slots):
                                nc.vector.tensor_scalar(
                                    out=S[:, t, :],
                                    in0=iota1[:],
                                    scalar1=dlb_sb[:, t0 + t : t0 + t + 1],
                                    scalar2=None,
                                    op0=mybir.AluOpType.is_equal,
                                )
                        nb_ps = nbps.tile([P, D2], f32, space="PSUM", tag="nbp",
                                          name=f"nbp{k}")
                        nmm = 1 if "segmm" in ablate else slots
                        for t in range(nmm):
                            nc.tensor.matmul(
                                nb_ps[:], lhsT=S[:, t, :], rhs=g[:, t, :],
                                start=(t == 0), stop=(t == nmm - 1),
                            )
                        # evacuate with 1/deg scaling on ScalarE
                        nc.scalar.activation(
                            nb_sb[:], nb_ps[:],
                            mybir.ActivationFunctionType.Copy,
                            scale=invd[:, k : k + 1],
                        )

                    xs_sb = wp.tile([P, D2], f32, tag="xs", bufs=GROUP + 2,
                                    name=f"xs{k}")
                    xs_t[k] = xs_sb
                    nc.sync.dma_start(xs_sb[:], d_xs[k * P : (k + 1) * P, :])
                    stx = wp.tile([P, 2, 6], f32, tag="stx", name=f"stx{k}")
                    stn = wp.tile([P, 2, 6], f32, tag="stn", name=f"stn{k}")
                    for b in range(2):
                        nc.vector.bn_stats(stx[:, b, :], xs_sb[:, b * D : (b + 1) * D])
                        nc.vector.bn_stats(stn[:, b, :], nb_sb[:, b * D : (b + 1) * D])
                    nc.vector.bn_aggr(mvg[:, 4 * gi + 0, :], stx[:, 0:1, :])
                    nc.vector.bn_aggr(mvg[:, 4 * gi + 1, :], stx[:, 1:2, :])
                    nc.vector.bn_aggr(mvg[:, 4 * gi + 2, :], stn[:, 0:1, :])
                    nc.vector.bn_aggr(mvg[:, 4 * gi + 3, :], stn[:, 1:2, :])

                # group-level rstd: one Sqrt (no act-table thrash) + reciprocal
                rsg = wp.tile([P, 4 * gs], f32, tag="rsg", bufs=2, name=f"rsg{k0}")
                nc.scalar.activation(
                    rsg[:], mvg[:, :, 1], mybir.ActivationFunctionType.Sqrt,
                    bias=eps_sb[:], scale=1.0,
                )
                nc.vector.reciprocal(rsg[:], rsg[:])

                # phase B: normalize + MLP + residual
                for gi in range(gs):
                    k = k0 + gi
                    nb_sb, xs_sb = nb_t[k], xs_t[k]
                    hx = wp.tile([P, D2], f32, tag="hx", name=f"hx{k}")
                    hn = wp.tile([P, D2], f32, tag="hn", name=f"hn{k}")
                    for b in range(2):
                        nc.vector.tensor_scalar(
                            out=hx[:, b * D : (b + 1) * D],
                            in0=xs_sb[:, b * D : (b + 1) * D],
                            scalar1=mvg[:, 4 * gi + b, 0:1],
                            scalar2=rsg[:, 4 * gi + b : 4 * gi + b + 1],
                            op0=mybir.AluOpType.subtract,
                            op1=mybir.AluOpType.mult,
                        )
                        nc.vector.tensor_scalar(
                            out=hn[:, b * D : (b + 1) * D],
                            in0=nb_sb[:, b * D : (b + 1) * D],
                            scalar1=mvg[:, 4 * gi + 2 + b, 0:1],
                            scalar2=rsg[:, 4 * gi + 2 + b : 4 * gi + 3 + b],
                            op0=mybir.AluOpType.subtract,
                            op1=mybir.AluOpType.mult,
                        )

                    # feature-major h via PE transposes; both batches side
                    # by side so mm1 streams N=256 in one f32r matmul
                    hTc = [hp.tile([P, D2], bf16, name=f"hTc{kt}_{k}",
                                   tag=f"hTc{kt}") for kt in range(2)]
                    for b in range(2):
                        for kt, srct in ((0, hx), (1, hn)):
                            tp = trps.tile([P, P], f32, space="PSUM", tag="tr",
                                           name=f"tr{b}{kt}_{k}")
                            nc.tensor.transpose(
                                tp[:], srct[:, b * D : (b + 1) * D], ident[:]
                            )
                            nc.scalar.copy(hTc[kt][:, b * D : (b + 1) * D], tp[:])

                    y_sb = wp.tile([P, D2], f32, tag="y", name=f"y{k}")
                    gsb = [hp.tile([P, D2], f32, name=f"gc{jt}_{k}",
                                   tag=f"gc{jt}") for jt in range(2)]
                    for jt in range(2):
                        m1 = mm1ps.tile([P, D2], f32, space="PSUM", tag="m1",
                                        name=f"m1_{jt}_{k}")
                        for kt in range(2):
                            nc.tensor.matmul(
                                m1[:], lhsT=w1s[kt][jt][:], rhs=hTc[kt][:],
                                start=(kt == 0), stop=(kt == 1),
                            )
                        nc.scalar.activation(
                            gsb[jt][:], m1[:],
                            mybir.ActivationFunctionType.Gelu,
                            bias=b1e_sb[:, jt : jt + 1], scale=1.0,
                        )
                    for b in range(2):
                        # y_psum = g^T @ W2 + I @ x + ones x b2  (node-major out)
                        m2 = mm2ps.tile([P, P], f32, space="PSUM", tag="m2",
                                        name=f"m2_{b}_{k}")
                        for kt in range(2):
                            nc.tensor.matmul(
                                m2[:], lhsT=gsb[kt][:, b * D : (b + 1) * D],
                                rhs=w2t[kt][:],
                                start=(kt == 0), stop=False,
                            )
                        nc.tensor.matmul(
                            m2[:], lhsT=ones1[:], rhs=b2r_sb[:],
                            start=False, stop=True,
                        )
                        nc.vector.tensor_add(
                            y_sb[:, b * D : (b + 1) * D], m2[:],
                            xs_sb[:, b * D : (b + 1) * D],
                        )
                    nc.sync.dma_start(d_y[k * P : (k + 1) * P, :], y_sb[:])
    nc.compile()
    return nc


def kernel(x, edge_src, edge_dst, degree, sn_g, sn_b, nn_g, nn_b, W1, b1, W2, b2):
    from concourse.bass_utils import run_bass_kernel_spmd

    x = np.asarray(x)
    Bb, N, D = x.shape
    assert Bb == 2 and D == P, (Bb, N, D)

    struct, percore, shared, tids, N = _prep(x, edge_src, edge_dst, degree)

    key = (struct["NTC"], tuple(struct["T0"]), tuple(struct["T1"]),
           struct["NA"], struct["NB"])
    if key not in _CACHE:
        _CACHE.clear()
        _CACHE[key] = _build(struct)
    nc = _CACHE[key]

    W1f = np.asarray(W1, dtype=np.float32)
    b1f = np.asarray(b1, dtype=np.float32).ravel()
    W2f = np.asarray(W2, dtype=np.float32)
    b2f = np.asarray(b2, dtype=np.float32).ravel()
    shared_map = dict(
        xpa=shared["xpa"],
        w1=np.ascontiguousarray(W1f),
        w2=np.ascontiguousarray(W2f),
        b1r=np.ascontiguousarray(b1f.reshape(2, P).T),
        b2r=np.ascontiguousarray(b2f.reshape(1, P)),
        gx=np.asarray(sn_g, np.float32).reshape(P, 1),
        gn=np.asarray(nn_g, np.float32).reshape(P, 1),
        bx=np.asarray(sn_b, np.float32).reshape(P, 1),
        bn=np.asarray(nn_b, np.float32).reshape(P, 1),
    )
    if shared["xpb"] is not None:
        shared_map["xpb"] = shared["xpb"]

    in_maps = []
    for c in range(NCORES):
        m = dict(shared_map)
        m["xs"] = np.ascontiguousarray(percore["xs"][c])
        m["idx"] = np.ascontiguousarray(percore["idx"][c])
        m["dlb"] = np.ascontiguousarray(percore["dlb"][c])
        m["deg"] = np.ascontiguousarray(percore["deg"][c])
        in_maps.append(m)

    res = run_bass_kernel_spmd(nc, in_maps, core_ids=list(range(NCORES)))

    y = np.empty((Bb, N, D), dtype=np.float32)
    NTC = struct["NTC"]
    for c in range(NCORES):
        yc = res.results[c]["y"]
        for k in range(NTC):
            t = tids[c][k]
            n0 = t * P
            n1 = min(n0 + P, N)
            if n1 <= n0:
                continue
            y[0, n0:n1, :] = yc[k * P : k * P + (n1 - n0), :D]
            y[1, n0:n1, :] = yc[k * P : k * P + (n1 - n0), D:]
    return y
